# revision 42
# baseline (speedup 1.0000x reference)
"""Trainium2 Bass kernel for nn_NeuralSurface (8-layer MLP SDF with harmonic
embedding + skip concat), data-parallel over 8 NeuronCores.

v3 layout strategy:
- Activations transposed in SBUF ([features, points]); weights stationary fp16;
  PE matmuls K/M-chunked to 128, N-tile NT=512 (one PSUM bank).
- K=39 embedding matmuls (layer 0 + layer 4's emb chunk) row-packed: even tile
  in array rows 0-63, odd tile in rows 64-127 via tile_position, running
  concurrently -> half the PE slots; host-side embedding args pack two tiles
  per column block (halves DMA + Sin work).
- Layer 0 of pair p+1 is software-pipelined into pair p (emitted after l4), so
  a pair starts at l1 with h0 already drained -> no pair-boundary PE bubble.
- Tile-outer MM order per layer ([A: m0c0,m0c1,m1c0,m1c1][B: ...]) gives every
  ReLU drain >=5 matmul-slots of cover before its consumer.
- Harmonic sin: host does the range reduction (ships ys = t - round(t) in fp16,
  packed two tiles per column block); on-chip it is one DMA + one ScalarE Sin.
- All weights ship in one DRAM tensor (one DMA) to avoid serialized
  DMA-issue latency at startup.
- ReLU drains alternate ACT/DVE by (t+m) parity; SDF finals split 1/1.
"""

import numpy as np

import concourse.bacc as bacc
import concourse.mybir as mybir
import concourse.tile as tile
from concourse.bass_utils import run_bass_kernel_spmd

AF = mybir.ActivationFunctionType
ALU = mybir.AluOpType
F32 = mybir.dt.float32
F16 = mybir.dt.float16

N_CORES = 8
N = 262144
NPC = N // N_CORES  # 32768 points per core
NT = 512  # points per n-tile (PSUM bank limit for fp32)
PAIRS = NPC // (2 * NT)  # 32
H = 256
E = 39
NHARM = 6
TWO_PI = float(2.0 * np.pi)

# Weight columns inside the packed weight tensors [128, *]:
# w0lo/w0hi are full-K copies of w0 with the unused row-half zeroed (tile A
# reads rows 0:39, tile B rows 64:103 of the packed emb tile; zero rows kill
# the other tile's contribution), so layer-0 matmuls keep the full-array
# config. w4e keeps the row-packed (64-row) form.
_WOFF = {}
_off = 0
for _name in ("w0lo", "w0hi", "w1a", "w1b", "w4ef", "w2a", "w2b", "w3a",
              "w3b", "w5a", "w5b", "w6a", "w6b", "w7a", "w7b", "w4a", "w4b"):
    _WOFF[_name] = _off
    _off += H
# wsdf chunks padded to M=128 (col 0 = wsdf, rest zero) so the SDF matmuls
# keep the full-array config -> LDWEIGHTS stays pipelined (M=1 config
# switches cost ~94ns each side).
_WOFF["wsdfa"] = _off
_WOFF["wsdfb"] = _off + 128
WCOLS = _off + 256

_CACHED = {}


def bass_ts(i, size):
    return slice(i * size, (i + 1) * size)


def _build():
    nc = bacc.Bacc("TRN2")

    ysh = nc.dram_tensor("ysh", [128, NPC // 2], F16, kind="ExternalInput").ap()
    ptseh = nc.dram_tensor("ptseh", [3, NPC // 2], F16, kind="ExternalInput").ap()
    ptsoh = nc.dram_tensor("ptsoh", [3, NPC // 2], F16, kind="ExternalInput").ap()
    # weights split: wbh1 carries what layers 0-1 need (arrives first), wbh2
    # the rest, so the first matmuls do not wait on one big transfer.
    W1COLS = 4 * H  # w0f, w4ef, w1a, w1b
    wbh1 = nc.dram_tensor("wbh1", [128, W1COLS], F16, kind="ExternalInput").ap()
    wbh2 = nc.dram_tensor("wbh2", [128, WCOLS - W1COLS], F16,
                          kind="ExternalInput").ap()
    bm17h = nc.dram_tensor("bm17h", [128, 17], F32, kind="ExternalInput").ap()
    out_o = nc.dram_tensor("out_o", [NPC // NT, NT], F32, kind="ExternalOutput").ap()

    with tile.TileContext(nc) as tc:
        with (
            tc.tile_pool(name="wp", bufs=1) as wp,
            tc.tile_pool(name="ep", bufs=3) as ep,
            tc.tile_pool(name="embp", bufs=3) as embp,
            tc.tile_pool(name="hp", bufs=6) as hp,
            tc.tile_pool(name="op", bufs=3) as op_,
            tc.tile_pool(name="pp", bufs=6, space="PSUM") as pp,
            tc.tile_pool(name="pf", bufs=1, space="PSUM") as pf,
        ):
            zcol = wp.tile([128, 1], F32, name="zcol")
            nc.vector.memset(zcol, 0.0)
            # dummy activation: forces the ACT table load (~1.3us) now,
            # instead of serialized behind the first ys DMA.
            sct = wp.tile([1, 1], F32, name="sct")
            nc.scalar.activation(
                sct, zcol[0:1, 0:1], AF.Sin, bias=zcol[0:1, 0:1], scale=1.0
            )

            def emit_emb_dma(p):
                # embedding args pair p: even tile rows 0:39, odd tile rows
                # 64:103; ys already range-reduced on host.
                ys = ep.tile([128, NT], F16, tag="ys")
                nc.sync.dma_start(out=ys, in_=ysh[:, bass_ts(p, NT)])
                return ys

            def emit_emb_sin(p, ys):
                emb = embp.tile([128, NT], F16, tag="emb")
                nc.scalar.activation(emb, ys, AF.Sin, bias=zcol, scale=TWO_PI)
                nc.sync.dma_start(out=emb[36:39, :], in_=ptseh[:, bass_ts(p, NT)])
                nc.sync.dma_start(out=emb[100:103, :], in_=ptsoh[:, bass_ts(p, NT)])
                return emb

            def emit_emb(p):
                return emit_emb_sin(p, emit_emb_dma(p))

            # DMA order: first-needed weights, emb args for pairs 0 and 1,
            # biases, then the bulk weights.
            wb1 = wp.tile_from(wbh1, name="wb1")
            ys0 = emit_emb_dma(0)
            bms = wp.tile_from(bm17h, name="bms")  # [128, 17] fp32
            ys1 = emit_emb_dma(1)
            wb2 = wp.tile_from(wbh2, name="wb2")
            W1COLS_ = 4 * H

            # HAM warmup: small matmuls gated on the wb1 DMA, bridging the
            # window between weight arrival and the first real matmul so the
            # PE clock gate is at 8/8 (and the pipeline hot) from the start.
            warm = pf.tile([1, 128], F32, tag="finA", name="warm")
            for _ in range(48):
                nc.tensor.matmul(
                    warm, wb1[:, 0:1], wb1[:, 0:128],
                    start=True, stop=True, skip_group_check=True,
                )

            def wcol(name, m=0):
                off = _WOFF[name] + m * 128
                if off < W1COLS_:
                    return wb1[:, off:off + 128]
                off -= W1COLS_
                return wb2[:, off:off + 128]

            def drain(li, t, m, ps, h):
                dst = h[:, bass_ts(2 * t + m, NT)]
                bias_ap = bms[:, li * 2 + m:li * 2 + m + 1]
                if (t + m + li) % 2 == 0:
                    nc.scalar.activation(dst, ps, AF.Relu, bias=bias_ap)
                else:
                    nc.vector.tensor_scalar(
                        dst, ps, bias_ap, 0.0, op0=ALU.add, op1=ALU.max
                    )

            def emit_l0_mms(emb):
                # layer 0: full-K matmuls against zero-padded weight copies
                # (w0lo kills rows 64:128, w0hi kills rows 0:64) -> no array
                # config switch. Own tag: h0 lives across the pair boundary.
                h = hp.tile([128, 4 * NT], F16, tag="h0")
                ps = {
                    (t, m): pp.tile([128, NT], F32, tag="ps", name="psmm")
                    for t in (0, 1) for m in (0, 1)
                }
                for t, wname in ((0, "w0lo"), (1, "w0hi")):
                    for m in (0, 1):
                        nc.tensor.matmul(
                            ps[(t, m)], wcol(wname, m), emb,
                            start=True, stop=True,
                        )
                return h, ps

            def emit_l0_drains(ps, h):
                for t in (0, 1):
                    for m in (0, 1):
                        drain(0, t, m, ps[(t, m)], h)

            def emit_layer(li, h_prev, mid=None):
                # layers 1,2,3,5,6,7: K=256 in 2 chunks, tile-outer order
                h = hp.tile([128, 4 * NT], F16, tag="h")
                ps = {
                    (t, m): pp.tile([128, NT], F32, tag="ps", name="psmm")
                    for t in (0, 1) for m in (0, 1)
                }
                for t in (0, 1):
                    for m in (0, 1):
                        for ci in (0, 1):
                            nc.tensor.matmul(
                                ps[(t, m)], wcol(f"w{li}{'ab'[ci]}", m),
                                h_prev[:, bass_ts(2 * t + ci, NT)],
                                start=(ci == 0), stop=(ci == 1),
                            )
                        drain(li, t, m, ps[(t, m)], h)
                    if t == 0 and mid is not None:
                        mid()
                return h

            def emit_l4(emb, h3):
                # layer 4: K = 39(emb, row-packed) + 256(h3, 2 full chunks)
                h = hp.tile([128, 4 * NT], F16, tag="h")
                ps = {
                    (t, m): pp.tile([128, NT], F32, tag="ps", name="psmm")
                    for t in (0, 1) for m in (0, 1)
                }
                for m in (0, 1):
                    nc.tensor.matmul(
                        ps[(0, m)], wcol("w4ef", m)[0:64, :], emb[0:64, :],
                        start=True, stop=False, tile_position=(0, 0),
                        skip_group_check=True,
                    )
                    nc.tensor.matmul(
                        ps[(1, m)], wcol("w4ef", m)[64:128, :], emb[64:128, :],
                        start=True, stop=False, tile_position=(64, 0),
                        skip_group_check=True,
                    )
                for t in (0, 1):
                    for m in (0, 1):
                        for ci, wname in ((0, "w4a"), (1, "w4b")):
                            nc.tensor.matmul(
                                ps[(t, m)], wcol(wname, m),
                                h3[:, bass_ts(2 * t + ci, NT)],
                                start=False, stop=(ci == 1),
                                skip_group_check=True,
                            )
                        drain(4, t, m, ps[(t, m)], h)
                return h

            def emit_sdf(p, h7):
                # final SDF layer: wsdf padded to M=128 (row 0 is the real
                # output) so the array config matches the layer matmuls and
                # LDWEIGHTS stays pipelined.
                psfa = pf.tile([128, NT], F32, tag="finA")
                psfb = pf.tile([128, NT], F32, tag="finB")
                for psf, t in ((psfa, 0), (psfb, 1)):
                    nc.tensor.matmul(
                        psf, wcol("wsdfa", 0), h7[:, bass_ts(2 * t, NT)],
                        start=True, stop=False,
                    )
                    nc.tensor.matmul(
                        psf, wcol("wsdfb", 0), h7[:, bass_ts(2 * t + 1, NT)],
                        start=False, stop=True,
                    )
                bsdf_ap = bms[0:1, 16:17]
                oa = op_.tile([1, NT], F32, tag="oa")
                nc.scalar.activation(oa, psfa[0:1, :], AF.Identity, bias=bsdf_ap)
                ob = op_.tile([1, NT], F32, tag="ob")
                nc.vector.tensor_scalar(
                    ob, psfb[0:1, :], bsdf_ap, 0.0, op0=ALU.add, op1=ALU.add
                )
                nc.sync.dma_start(out=out_o[2 * p:2 * p + 1, :], in_=oa)
                nc.sync.dma_start(out=out_o[2 * p + 1:2 * p + 2, :], in_=ob)

            # ---- main pipeline ----
            emb_cur = emit_emb_sin(0, ys0)
            h0_cur, ps0 = emit_l0_mms(emb_cur)
            emit_l0_drains(ps0, h0_cur)
            emb_next = emit_emb_sin(1, ys1)
            for p in range(PAIRS):
                # l0 of the next pair leads the pair: it is independent of
                # l1..l7(p) (h0(p) was drained last pair), and its PSUM banks
                # recycle before l2 needs the ring slots. Exception pair 0:
                # emb(1) is still in flight, so l0n goes after l2 to not
                # block l1(0) in the PE queue.
                if emb_next is not None and p > 0:
                    h0_next, ps0n = emit_l0_mms(emb_next)
                    emit_l0_drains(ps0n, h0_next)
                # prefetch the embedding-args DMA two pairs ahead; its Sin +
                # pts DMAs run mid-pair (after l4) where ACT has slack.
                ys_next2 = emit_emb_dma(p + 2) if p + 2 < PAIRS else None
                h1 = emit_layer(1, h0_cur)
                if emb_next is not None and p == 0:
                    # after l1 so pair 0's ring keeps the steady-state
                    # l4-reuses-l2/l3 distance
                    h0_next, ps0n = emit_l0_mms(emb_next)
                    emit_l0_drains(ps0n, h0_next)
                h2 = emit_layer(2, h1)
                h3 = emit_layer(3, h2)
                h4 = emit_l4(emb_cur, h3)
                emb_next2 = (
                    emit_emb_sin(p + 2, ys_next2) if ys_next2 is not None else None
                )
                h5 = emit_layer(5, h4)
                h6 = emit_layer(6, h5)
                h7 = emit_layer(7, h6)
                emit_sdf(p, h7)
                if emb_next is not None:
                    emb_cur, h0_cur = emb_next, h0_next
                    emb_next = emb_next2
    nc.compile()
    return nc


def _prep_maps(points, ws, bs, wsdf, bsdf):
    pts = np.ascontiguousarray(points, dtype=np.float32).reshape(N, 3)
    freqs = (2.0 ** np.arange(NHARM)).astype(np.float32)
    fcol18 = (np.repeat(freqs[None, :], 3, axis=0).reshape(18, 1) / TWO_PI).astype(
        np.float32
    )

    bm17 = np.zeros((128, 17), dtype=np.float32)
    for i in range(8):
        for m in range(2):
            bm17[:, i * 2 + m] = bs[i][m * 128:(m + 1) * 128]
    bm17[:, 16] = float(np.ravel(bsdf)[0])

    wb = np.zeros((128, WCOLS), dtype=np.float16)
    wb[0:E, _WOFF["w0lo"]:_WOFF["w0lo"] + H] = ws[0].astype(np.float16)
    wb[64:64 + E, _WOFF["w0hi"]:_WOFF["w0hi"] + H] = ws[0].astype(np.float16)
    wb[0:E, _WOFF["w4ef"]:_WOFF["w4ef"] + H] = ws[4][0:E].astype(np.float16)
    wb[64:64 + E, _WOFF["w4ef"]:_WOFF["w4ef"] + H] = ws[4][0:E].astype(np.float16)
    for i in (1, 2, 3, 5, 6, 7):
        wb[:, _WOFF[f"w{i}a"]:_WOFF[f"w{i}a"] + H] = ws[i][0:128].astype(np.float16)
        wb[:, _WOFF[f"w{i}b"]:_WOFF[f"w{i}b"] + H] = ws[i][128:256].astype(np.float16)
    wb[:, _WOFF["w4a"]:_WOFF["w4a"] + H] = ws[4][E:E + 128].astype(np.float16)
    wb[:, _WOFF["w4b"]:_WOFF["w4b"] + H] = ws[4][E + 128:E + 256].astype(np.float16)
    wb[:, _WOFF["wsdfa"]] = wsdf[0:128, 0].astype(np.float16)  # col 0; rest 0
    wb[:, _WOFF["wsdfb"]] = wsdf[128:256, 0].astype(np.float16)

    common = {
        "wbh1": np.ascontiguousarray(wb[:, :4 * H]),
        "wbh2": np.ascontiguousarray(wb[:, 4 * H:]),
        "bm17h": bm17,
    }

    in_maps = []
    for c in range(N_CORES):
        sl = pts[c * NPC:(c + 1) * NPC]  # [NPC, 3]
        ptsT = np.ascontiguousarray(sl.T)  # [3, NPC]
        rep3 = np.repeat(ptsT, NHARM, axis=0)  # [18, NPC]
        t18 = rep3 * fcol18  # x * 2^j / (2pi), exact fp32 scaling
        t18c = t18 + np.float32(0.25)  # cos rows as shifted sin
        ys18 = (t18 - np.rint(t18)).astype(np.float32)
        ys18c = (t18c - np.rint(t18c)).astype(np.float32)
        # pack: even tile -> rows 0:36, odd tile -> rows 64:100
        ev_s = ys18.reshape(18, PAIRS, 2, NT)[:, :, 0, :].reshape(18, NPC // 2)
        od_s = ys18.reshape(18, PAIRS, 2, NT)[:, :, 1, :].reshape(18, NPC // 2)
        ev_c = ys18c.reshape(18, PAIRS, 2, NT)[:, :, 0, :].reshape(18, NPC // 2)
        od_c = ys18c.reshape(18, PAIRS, 2, NT)[:, :, 1, :].reshape(18, NPC // 2)
        ysh = np.zeros((128, NPC // 2), dtype=np.float16)
        ysh[0:18], ysh[18:36] = ev_s, ev_c
        ysh[64:82], ysh[82:100] = od_s, od_c
        ptsv = ptsT.reshape(3, PAIRS, 2, NT)
        m = dict(common)
        m["ysh"] = ysh
        m["ptseh"] = np.ascontiguousarray(
            ptsv[:, :, 0, :].reshape(3, NPC // 2)
        ).astype(np.float16)
        m["ptsoh"] = np.ascontiguousarray(
            ptsv[:, :, 1, :].reshape(3, NPC // 2)
        ).astype(np.float16)
        in_maps.append(m)
    return in_maps


def kernel(
    points, w0, b0, w1, b1, w2, b2, w3, b3, w4, b4, w5, b5, w6, b6, w7, b7,
    wsdf, bsdf,
):
    ws = [np.asarray(w, dtype=np.float32) for w in (w0, w1, w2, w3, w4, w5, w6, w7)]
    bs = [np.asarray(b, dtype=np.float32) for b in (b0, b1, b2, b3, b4, b5, b6, b7)]
    in_maps = _prep_maps(
        np.asarray(points), ws, bs,
        np.asarray(wsdf, dtype=np.float32), np.asarray(bsdf, dtype=np.float32),
    )

    if "nc" not in _CACHED:
        _CACHED["nc"] = _build()
    nc = _CACHED["nc"]

    res = run_bass_kernel_spmd(nc, in_maps, core_ids=list(range(N_CORES)))
    out = np.concatenate(
        [res.results[c]["out_o"] for c in range(N_CORES)], axis=0
    ).reshape(N, 1).astype(np.float32)
    return out


# revision 43
# speedup vs baseline: 1.2308x; 1.2308x over previous
"""Trainium2 Bass kernel for nn_NeuralSurface (8-layer MLP SDF with harmonic
embedding + skip concat), data-parallel over 8 NeuronCores.

v3 layout strategy:
- Activations transposed in SBUF ([features, points]); weights stationary fp16;
  PE matmuls K/M-chunked to 128, N-tile NT=512 (one PSUM bank).
- K=39 embedding matmuls (layer 0 + layer 4's emb chunk) row-packed: even tile
  in array rows 0-63, odd tile in rows 64-127 via tile_position, running
  concurrently -> half the PE slots; host-side embedding args pack two tiles
  per column block (halves DMA + Sin work).
- Layer 0 of pair p+1 is software-pipelined into pair p (emitted after l4), so
  a pair starts at l1 with h0 already drained -> no pair-boundary PE bubble.
- Tile-outer MM order per layer ([A: m0c0,m0c1,m1c0,m1c1][B: ...]) gives every
  ReLU drain >=5 matmul-slots of cover before its consumer.
- Harmonic sin: host does the range reduction (ships ys = t - round(t) in fp16,
  packed two tiles per column block); on-chip it is one DMA + one ScalarE Sin.
- All weights ship in one DRAM tensor (one DMA) to avoid serialized
  DMA-issue latency at startup.
- ReLU drains alternate ACT/DVE by (t+m) parity; SDF finals split 1/1.
"""

import numpy as np

import concourse.bacc as bacc
import concourse.mybir as mybir
import concourse.tile as tile
from concourse.bass_utils import run_bass_kernel_spmd

AF = mybir.ActivationFunctionType
ALU = mybir.AluOpType
F32 = mybir.dt.float32
F16 = mybir.dt.float16

N_CORES = 8
N = 262144
NPC = N // N_CORES  # 32768 points per core
NT = 512  # points per n-tile (PSUM bank limit for fp32)
PAIRS = NPC // (2 * NT)  # 32
H = 256
E = 39
NHARM = 6
TWO_PI = float(2.0 * np.pi)

# Weight columns inside the packed weight tensors [128, *]:
# w0lo/w0hi are full-K copies of w0 with the unused row-half zeroed (tile A
# reads rows 0:39, tile B rows 64:103 of the packed emb tile; zero rows kill
# the other tile's contribution), so layer-0 matmuls keep the full-array
# config. w4e keeps the row-packed (64-row) form.
_WOFF = {}
_off = 0
for _name in ("w0lo", "w0hi", "w1a", "w1b", "w4ef", "w2a", "w2b", "w3a",
              "w3b", "w5a", "w5b", "w6a", "w6b", "w7a", "w7b", "w4a", "w4b"):
    _WOFF[_name] = _off
    _off += H
# wsdf chunks padded to M=128 (col 0 = wsdf, rest zero) so the SDF matmuls
# keep the full-array config -> LDWEIGHTS stays pipelined (M=1 config
# switches cost ~94ns each side).
_WOFF["wsdfa"] = _off
_WOFF["wsdfb"] = _off + 128
WCOLS = _off + 256

_CACHED = {}


def bass_ts(i, size):
    return slice(i * size, (i + 1) * size)


def _build():
    nc = bacc.Bacc("TRN2")

    ysh = nc.dram_tensor("ysh", [128, NPC // 2], F16, kind="ExternalInput").ap()
    ptseh = nc.dram_tensor("ptseh", [3, NPC // 2], F16, kind="ExternalInput").ap()
    ptsoh = nc.dram_tensor("ptsoh", [3, NPC // 2], F16, kind="ExternalInput").ap()
    # weights split: wbh1 carries what layers 0-1 need (arrives first), wbh2
    # the rest, so the first matmuls do not wait on one big transfer.
    W1COLS = 4 * H  # w0f, w4ef, w1a, w1b
    wbh1 = nc.dram_tensor("wbh1", [128, W1COLS], F16, kind="ExternalInput").ap()
    wbh2 = nc.dram_tensor("wbh2", [128, WCOLS - W1COLS], F16,
                          kind="ExternalInput").ap()
    bm17h = nc.dram_tensor("bm17h", [128, 17], F32, kind="ExternalInput").ap()
    out_o = nc.dram_tensor("out_o", [NPC // NT, NT], F32, kind="ExternalOutput").ap()

    with tile.TileContext(nc) as tc:
        with (
            tc.tile_pool(name="wp", bufs=1) as wp,
            tc.tile_pool(name="ep", bufs=3) as ep,
            tc.tile_pool(name="embp", bufs=3) as embp,
            tc.tile_pool(name="hp", bufs=6) as hp,
            tc.tile_pool(name="op", bufs=3) as op_,
            tc.tile_pool(name="pp", bufs=6, space="PSUM") as pp,
            tc.tile_pool(name="pf", bufs=1, space="PSUM") as pf,
        ):
            zcol = wp.tile([128, 1], F32, name="zcol")
            nc.vector.memset(zcol, 0.0)
            # dummy activation: forces the ACT table load (~1.3us) now,
            # instead of serialized behind the first ys DMA.
            sct = wp.tile([1, 1], F32, name="sct")
            nc.scalar.activation(
                sct, zcol[0:1, 0:1], AF.Sin, bias=zcol[0:1, 0:1], scale=1.0
            )

            def emit_emb_dma(p):
                # embedding args pair p: even tile rows 0:39, odd tile rows
                # 64:103; ys already range-reduced on host.
                ys = ep.tile([128, NT], F16, tag="ys")
                nc.sync.dma_start(out=ys, in_=ysh[:, bass_ts(p, NT)])
                return ys

            def emit_emb_sin(p, ys):
                emb = embp.tile([128, NT], F16, tag="emb")
                nc.scalar.activation(emb, ys, AF.Sin, bias=zcol, scale=TWO_PI)
                nc.sync.dma_start(out=emb[36:39, :], in_=ptseh[:, bass_ts(p, NT)])
                nc.sync.dma_start(out=emb[100:103, :], in_=ptsoh[:, bass_ts(p, NT)])
                return emb

            def emit_emb(p):
                return emit_emb_sin(p, emit_emb_dma(p))

            # DMA order: first-needed weights, emb args for pairs 0 and 1,
            # biases, then the bulk weights.
            wb1 = wp.tile_from(wbh1, name="wb1")
            ys0 = emit_emb_dma(0)
            bms = wp.tile_from(bm17h, name="bms")  # [128, 17] fp32
            ys1 = emit_emb_dma(1)
            wb2 = wp.tile_from(wbh2, name="wb2")
            W1COLS_ = 4 * H

            # HAM warmup: small matmuls gated on the wb1 DMA, bridging the
            # window between weight arrival and the first real matmul so the
            # PE clock gate is at 8/8 (and the pipeline hot) from the start.
            warm = pf.tile([1, 128], F32, tag="finA", name="warm")
            for _ in range(48):
                nc.tensor.matmul(
                    warm, wb1[:, 0:1], wb1[:, 0:128],
                    start=True, stop=True, skip_group_check=True,
                )

            def wcol(name, m=0):
                off = _WOFF[name] + m * 128
                if off < W1COLS_:
                    return wb1[:, off:off + 128]
                off -= W1COLS_
                return wb2[:, off:off + 128]

            def drain(li, t, m, ps, h):
                dst = h[:, bass_ts(2 * t + m, NT)]
                bias_ap = bms[:, li * 2 + m:li * 2 + m + 1]
                if (t + m) % 2 == 0:
                    nc.scalar.activation(dst, ps, AF.Relu, bias=bias_ap)
                else:
                    nc.vector.tensor_scalar(
                        dst, ps, bias_ap, 0.0, op0=ALU.add, op1=ALU.max
                    )

            def emit_l0_mms(emb):
                # layer 0: full-K matmuls against zero-padded weight copies
                # (w0lo kills rows 64:128, w0hi kills rows 0:64) -> no array
                # config switch. Own tag: h0 lives across the pair boundary.
                h = hp.tile([128, 4 * NT], F16, tag="h0")
                ps = {
                    (t, m): pp.tile([128, NT], F32, tag="ps", name="psmm")
                    for t in (0, 1) for m in (0, 1)
                }
                for t, wname in ((0, "w0lo"), (1, "w0hi")):
                    for m in (0, 1):
                        nc.tensor.matmul(
                            ps[(t, m)], wcol(wname, m), emb,
                            start=True, stop=True,
                        )
                return h, ps

            def emit_l0_drains(ps, h):
                for t in (0, 1):
                    for m in (0, 1):
                        drain(0, t, m, ps[(t, m)], h)

            def emit_layer(li, h_prev, mid=None):
                # layers 1,2,3,5,6,7: K=256 in 2 chunks, tile-outer order
                h = hp.tile([128, 4 * NT], F16, tag="h")
                ps = {
                    (t, m): pp.tile([128, NT], F32, tag="ps", name="psmm")
                    for t in (0, 1) for m in (0, 1)
                }
                for t in (0, 1):
                    for m in (0, 1):
                        for ci in (0, 1):
                            nc.tensor.matmul(
                                ps[(t, m)], wcol(f"w{li}{'ab'[ci]}", m),
                                h_prev[:, bass_ts(2 * t + ci, NT)],
                                start=(ci == 0), stop=(ci == 1),
                            )
                        drain(li, t, m, ps[(t, m)], h)
                    if t == 0 and mid is not None:
                        mid()
                return h

            def emit_l4(emb, h3):
                # layer 4: K = 39(emb, row-packed) + 256(h3, 2 full chunks)
                h = hp.tile([128, 4 * NT], F16, tag="h")
                ps = {
                    (t, m): pp.tile([128, NT], F32, tag="ps", name="psmm")
                    for t in (0, 1) for m in (0, 1)
                }
                for m in (0, 1):
                    nc.tensor.matmul(
                        ps[(0, m)], wcol("w4ef", m)[0:64, :], emb[0:64, :],
                        start=True, stop=False, tile_position=(0, 0),
                        skip_group_check=True,
                    )
                    nc.tensor.matmul(
                        ps[(1, m)], wcol("w4ef", m)[64:128, :], emb[64:128, :],
                        start=True, stop=False, tile_position=(64, 0),
                        skip_group_check=True,
                    )
                for t in (0, 1):
                    for m in (0, 1):
                        for ci, wname in ((0, "w4a"), (1, "w4b")):
                            nc.tensor.matmul(
                                ps[(t, m)], wcol(wname, m),
                                h3[:, bass_ts(2 * t + ci, NT)],
                                start=False, stop=(ci == 1),
                                skip_group_check=True,
                            )
                        drain(4, t, m, ps[(t, m)], h)
                return h

            def emit_sdf(p, h7):
                # final SDF layer: wsdf padded to M=128 (row 0 is the real
                # output) so the array config matches the layer matmuls and
                # LDWEIGHTS stays pipelined.
                psfa = pf.tile([128, NT], F32, tag="finA")
                psfb = pf.tile([128, NT], F32, tag="finB")
                for psf, t in ((psfa, 0), (psfb, 1)):
                    nc.tensor.matmul(
                        psf, wcol("wsdfa", 0), h7[:, bass_ts(2 * t, NT)],
                        start=True, stop=False,
                    )
                    nc.tensor.matmul(
                        psf, wcol("wsdfb", 0), h7[:, bass_ts(2 * t + 1, NT)],
                        start=False, stop=True,
                    )
                bsdf_ap = bms[0:1, 16:17]
                oa = op_.tile([1, NT], F32, tag="oa")
                nc.scalar.activation(oa, psfa[0:1, :], AF.Identity, bias=bsdf_ap)
                ob = op_.tile([1, NT], F32, tag="ob")
                nc.vector.tensor_scalar(
                    ob, psfb[0:1, :], bsdf_ap, 0.0, op0=ALU.add, op1=ALU.add
                )
                nc.sync.dma_start(out=out_o[2 * p:2 * p + 1, :], in_=oa)
                nc.sync.dma_start(out=out_o[2 * p + 1:2 * p + 2, :], in_=ob)

            # ---- main pipeline ----
            emb_cur = emit_emb_sin(0, ys0)
            h0_cur, ps0 = emit_l0_mms(emb_cur)
            emit_l0_drains(ps0, h0_cur)
            emb_next = emit_emb_sin(1, ys1)
            for p in range(PAIRS):
                # l0 of the next pair leads the pair: it is independent of
                # l1..l7(p) (h0(p) was drained last pair), and its PSUM banks
                # recycle before l2 needs the ring slots. Exception pair 0:
                # emb(1) is still in flight, so l0n goes after l2 to not
                # block l1(0) in the PE queue.
                if emb_next is not None and p > 0:
                    h0_next, ps0n = emit_l0_mms(emb_next)
                    emit_l0_drains(ps0n, h0_next)
                # prefetch the embedding-args DMA two pairs ahead; its Sin +
                # pts DMAs run mid-pair (after l4) where ACT has slack.
                ys_next2 = emit_emb_dma(p + 2) if p + 2 < PAIRS else None
                h1 = emit_layer(1, h0_cur)
                if emb_next is not None and p == 0:
                    # after l1 so pair 0's ring keeps the steady-state
                    # l4-reuses-l2/l3 distance
                    h0_next, ps0n = emit_l0_mms(emb_next)
                    emit_l0_drains(ps0n, h0_next)
                h2 = emit_layer(2, h1)
                h3 = emit_layer(3, h2)
                h4 = emit_l4(emb_cur, h3)
                emb_next2 = (
                    emit_emb_sin(p + 2, ys_next2) if ys_next2 is not None else None
                )
                h5 = emit_layer(5, h4)
                h6 = emit_layer(6, h5)
                h7 = emit_layer(7, h6)
                emit_sdf(p, h7)
                if emb_next is not None:
                    emb_cur, h0_cur = emb_next, h0_next
                    emb_next = emb_next2
    nc.compile()
    return nc


def _prep_maps(points, ws, bs, wsdf, bsdf):
    pts = np.ascontiguousarray(points, dtype=np.float32).reshape(N, 3)
    freqs = (2.0 ** np.arange(NHARM)).astype(np.float32)
    fcol18 = (np.repeat(freqs[None, :], 3, axis=0).reshape(18, 1) / TWO_PI).astype(
        np.float32
    )

    bm17 = np.zeros((128, 17), dtype=np.float32)
    for i in range(8):
        for m in range(2):
            bm17[:, i * 2 + m] = bs[i][m * 128:(m + 1) * 128]
    bm17[:, 16] = float(np.ravel(bsdf)[0])

    wb = np.zeros((128, WCOLS), dtype=np.float16)
    wb[0:E, _WOFF["w0lo"]:_WOFF["w0lo"] + H] = ws[0].astype(np.float16)
    wb[64:64 + E, _WOFF["w0hi"]:_WOFF["w0hi"] + H] = ws[0].astype(np.float16)
    wb[0:E, _WOFF["w4ef"]:_WOFF["w4ef"] + H] = ws[4][0:E].astype(np.float16)
    wb[64:64 + E, _WOFF["w4ef"]:_WOFF["w4ef"] + H] = ws[4][0:E].astype(np.float16)
    for i in (1, 2, 3, 5, 6, 7):
        wb[:, _WOFF[f"w{i}a"]:_WOFF[f"w{i}a"] + H] = ws[i][0:128].astype(np.float16)
        wb[:, _WOFF[f"w{i}b"]:_WOFF[f"w{i}b"] + H] = ws[i][128:256].astype(np.float16)
    wb[:, _WOFF["w4a"]:_WOFF["w4a"] + H] = ws[4][E:E + 128].astype(np.float16)
    wb[:, _WOFF["w4b"]:_WOFF["w4b"] + H] = ws[4][E + 128:E + 256].astype(np.float16)
    wb[:, _WOFF["wsdfa"]] = wsdf[0:128, 0].astype(np.float16)  # col 0; rest 0
    wb[:, _WOFF["wsdfb"]] = wsdf[128:256, 0].astype(np.float16)

    common = {
        "wbh1": np.ascontiguousarray(wb[:, :4 * H]),
        "wbh2": np.ascontiguousarray(wb[:, 4 * H:]),
        "bm17h": bm17,
    }

    in_maps = []
    for c in range(N_CORES):
        sl = pts[c * NPC:(c + 1) * NPC]  # [NPC, 3]
        ptsT = np.ascontiguousarray(sl.T)  # [3, NPC]
        rep3 = np.repeat(ptsT, NHARM, axis=0)  # [18, NPC]
        t18 = rep3 * fcol18  # x * 2^j / (2pi), exact fp32 scaling
        t18c = t18 + np.float32(0.25)  # cos rows as shifted sin
        ys18 = (t18 - np.rint(t18)).astype(np.float32)
        ys18c = (t18c - np.rint(t18c)).astype(np.float32)
        # pack: even tile -> rows 0:36, odd tile -> rows 64:100
        ev_s = ys18.reshape(18, PAIRS, 2, NT)[:, :, 0, :].reshape(18, NPC // 2)
        od_s = ys18.reshape(18, PAIRS, 2, NT)[:, :, 1, :].reshape(18, NPC // 2)
        ev_c = ys18c.reshape(18, PAIRS, 2, NT)[:, :, 0, :].reshape(18, NPC // 2)
        od_c = ys18c.reshape(18, PAIRS, 2, NT)[:, :, 1, :].reshape(18, NPC // 2)
        ysh = np.zeros((128, NPC // 2), dtype=np.float16)
        ysh[0:18], ysh[18:36] = ev_s, ev_c
        ysh[64:82], ysh[82:100] = od_s, od_c
        ptsv = ptsT.reshape(3, PAIRS, 2, NT)
        m = dict(common)
        m["ysh"] = ysh
        m["ptseh"] = np.ascontiguousarray(
            ptsv[:, :, 0, :].reshape(3, NPC // 2)
        ).astype(np.float16)
        m["ptsoh"] = np.ascontiguousarray(
            ptsv[:, :, 1, :].reshape(3, NPC // 2)
        ).astype(np.float16)
        in_maps.append(m)
    return in_maps


def kernel(
    points, w0, b0, w1, b1, w2, b2, w3, b3, w4, b4, w5, b5, w6, b6, w7, b7,
    wsdf, bsdf,
):
    ws = [np.asarray(w, dtype=np.float32) for w in (w0, w1, w2, w3, w4, w5, w6, w7)]
    bs = [np.asarray(b, dtype=np.float32) for b in (b0, b1, b2, b3, b4, b5, b6, b7)]
    in_maps = _prep_maps(
        np.asarray(points), ws, bs,
        np.asarray(wsdf, dtype=np.float32), np.asarray(bsdf, dtype=np.float32),
    )

    if "nc" not in _CACHED:
        _CACHED["nc"] = _build()
    nc = _CACHED["nc"]

    res = run_bass_kernel_spmd(nc, in_maps, core_ids=list(range(N_CORES)))
    out = np.concatenate(
        [res.results[c]["out_o"] for c in range(N_CORES)], axis=0
    ).reshape(N, 1).astype(np.float32)
    return out
